# revision 5
# baseline (speedup 1.0000x reference)
"""AngularContrastiveLoss v4: symmetric half-band kernel, 8 cores.

v3 + stall fixes: slab/guard on a dedicated 1-bank psum pool (the stream's
double-buffer is never blocked by DVE consumers), diagonal masked by a PE
zero-matmul instead of a DVE mul (no cross-engine round trip inside the
stream), 512-wide first chunk + split input DMAs for a fast start, colsum
pairs and guard pieces spread between stream chunks via a filler queue,
tail colsums emitted as soon as the covering t7 chunk is exp'd.
Math identical to kernel3 (see its docstring); host assembly identical
except guard maxes arrive raw (20 pieces) and are scaled by rn on host.
"""
import numpy as np

import concourse.bass as bass
import concourse.bacc as bacc
import concourse.mybir as mybir
from concourse.tile import TileContext
from concourse.bass_utils import run_bass_kernel_spmd

F32 = mybir.dt.float32
BF16 = mybir.dt.bfloat16
FP8 = mybir.dt.float8e4
FP8W = mybir.dt.float8e5
PM = mybir.MatmulPerfMode
AF = mybir.ActivationFunctionType
OP = mybir.AluOpType
AX = mybir.AxisListType

NCORES = 8
N = 4096
DA = 256
NCLS = 50
T = 0.06
MARGIN = 0.5
HALF_PI = float(np.float32(np.pi / 2))
INV_T = float(np.float32(1.0 / T))
NEG5LN2 = float(np.float32(-5.0 * np.log(2.0)))
XSCALE = 32.0
SHARD = N // NCORES
NBIN = 7
SLABW = NBIN * 128
CW = 1536
ROWC = 4608
POLY_SAFE = 0.45
GUARD_SAFE = 0.80

CHUNKS0 = [(0, 512), (512, 1536), (2048, 1536), (3584, 1024)]
CHUNKS = [(0, 1536), (1536, 1536), (3072, 1536)]
NSLOT = 4 + 7 * 3  # 25 exp accum slots


def _slots(t):
    return list(range(4)) if t == 0 else [4 + 3 * (t - 1) + c for c in range(3)]


def _chunks(t):
    return CHUNKS0 if t == 0 else CHUNKS


def _exports(t):
    if t < 4:
        e = [(c1 - 1, 512 * c1) for c1 in (1, 2, 3)]
        e += [(4 + c2, 2560 + 512 * c2) for c2 in (0, 1, 2, 3)]
    else:
        e = [(c1 - 1, 512 * c1 - 512) for c1 in (1, 2, 3, 4)]
        e += [(4 + c2, 2048 + 512 * c2) for c2 in (1, 2, 3)]
    return e


# guard piece schedule: piece g = 5*it + pc; cols [512pc, 512pc+512)
GUARD_SCHED = {1: [0, 5, 10], 2: [15, 1, 6], 3: [11, 16, 2],
               4: [7, 12, 17, 3], 5: [8, 13, 14, 18], 6: [4, 9, 19]}


def build():
    nc = bacc.Bacc("TRN2", target_bir_lowering=False, debug=False,
                   num_devices=NCORES)
    cftd = nc.declare_dram_parameter("cfT", [128, 5120], BF16, isOutput=False)
    amtd = nc.declare_dram_parameter("amT", [DA, 2560], FP8, isOutput=False)
    slbd = nc.declare_dram_parameter("slabT", [DA, SLABW], BF16, isOutput=False)
    mskd = nc.declare_dram_parameter("l2mask", [128, SLABW], BF16, isOutput=False)
    ax8d = nc.declare_dram_parameter("aux8", [128, 8, 2, 128], FP8W, isOutput=False)
    out1d = nc.declare_dram_parameter("out1", [128, 48], F32, isOutput=True)
    out2d = nc.declare_dram_parameter("out2", [8, 512], F32, isOutput=True)

    with TileContext(nc) as tc:
        with tc.sbuf_pool(name="persist", bufs=1) as PP:
            cfT = PP.tile([128, 5120], BF16, tag="cfT")
            amT = PP.tile([128, 2, 2560], FP8, tag="amT")
            slabT = PP.tile([128, 2, SLABW], BF16, tag="slabT")
            l2m = PP.tile([128, SLABW], BF16, tag="l2m")
            ohm = PP.tile([128, 8, 2, 128], FP8W, tag="ohm")
            zer = PP.tile([128, 128], BF16, tag="zer")
            X = PP.tile([128, 8, ROWC], FP8W, tag="X")
            out1 = PP.tile([128, 48], F32, tag="out1")
            dacc = out1[:, 0:NSLOT]
            gt = out1[:, NSLOT:NSLOT + 20]
            hp = PP.tile([128, 1], F32, tag="hp")
            b5 = PP.tile([128, 1], F32, tag="b5")
            warm = PP.tile([128, 1], F32, tag="warm")
            cso = PP.tile([8, 512], F32, tag="cso")

            nc.vector.memset(b5, NEG5LN2)
            nc.vector.memset(hp, HALF_PI)
            nc.vector.memset(zer, 0.0)
            nc.scalar.activation(warm, hp, AF.Exp)

            nc.sync.dma_start(out=cfT[:, 0:512], in_=cftd[:, 0:512])
            nc.sync.dma_start(out=cfT[:, 512:1536], in_=cftd[:, 512:1536])
            nc.sync.dma_start(out=cfT[:, 1536:2048], in_=cftd[:, 1536:2048])
            nc.sync.dma_start(out=cfT[:, 2048:3584], in_=cftd[:, 2048:3584])
            nc.sync.dma_start(out=ohm, in_=ax8d[:, :, :, :])
            nc.sync.dma_start(
                out=amT[:, :, 0:1536],
                in_=amtd[:, 0:1536].rearrange("(c p) w -> p c w", p=128))
            nc.sync.dma_start(out=cfT[:, 3584:5120], in_=cftd[:, 3584:5120])
            nc.sync.dma_start(
                out=slabT,
                in_=slbd[:, :].rearrange("(c p) w -> p c w", p=128))
            nc.sync.dma_start(
                out=amT[:, :, 1536:2560],
                in_=amtd[:, 1536:2560].rearrange("(c p) w -> p c w", p=128))
            nc.sync.dma_start(out=l2m, in_=mskd[:, :])

            with tc.psum_pool(name="stream", bufs=2) as SP, \
                 tc.psum_pool(name="cspool", bufs=1) as CS, \
                 tc.psum_pool(name="auxp", bufs=1) as AXP, \
                 tc.sbuf_pool(name="slw", bufs=2) as SW:

                csum = CS.tile([128, 512], F32, tag="csum")
                cs_state = {"first": True}

                def cs_mm(t, m, off, last=False):
                    nc.tensor.matmul(
                        csum, ohm[:, m, :, :], X[:, t:t + 2, off:off + 512],
                        start=cs_state["first"], stop=last,
                        perf_mode=PM.DoubleRow, skip_group_check=True)
                    cs_state["first"] = False

                def chunk(t, c0, w, slot, fillers, max_pop=2):
                    base = 0 if t < 4 else 512
                    lt = 128 * t if t < 4 else 2560 + 128 * (t - 4)
                    lhs = cfT[:, lt:lt + 128]
                    dcol = lt - base
                    ps = SP.tile([128, CW], F32, tag="ps")
                    for s in range(w // 512):
                        nc.tensor.matmul(
                            ps[:, 512 * s:512 * s + 512], lhs,
                            cfT[:, base + c0 + 512 * s:base + c0 + 512 * s + 512],
                            start=True, stop=True)
                    if c0 <= dcol < c0 + w:
                        off = dcol - c0
                        nc.tensor.matmul(ps[:, off:off + 128], lhs, zer,
                                         start=True, stop=True,
                                         skip_group_check=True)
                    n = 0
                    while fillers and n < max_pop:
                        fillers.pop(0)()
                        n += 1
                    nc.scalar.activation(
                        X[:, t, c0:c0 + w], ps[:, 0:w], AF.Exp,
                        scale=INV_T, bias=b5[:, 0:1],
                        accum_out=dacc[:, slot:slot + 1])

                def guard_piece(g):
                    it, pc = g // 5, g % 5
                    gp = AXP.tile([128, 512], F32, tag="axp")
                    nc.tensor.matmul(
                        gp, amT[:, :, 128 * it:128 * it + 128],
                        amT[:, :, 512 * pc:512 * pc + 512],
                        start=True, stop=True, perf_mode=PM.DoubleRow)
                    if pc == 0:
                        off = 128 * it
                        nc.tensor.matmul(gp[:, off:off + 128],
                                         cfT[:, 0:128], zer,
                                         start=True, stop=True,
                                         skip_group_check=True)
                    nc.vector.tensor_reduce(gt[:, g:g + 1], gp, axis=AX.X,
                                            op=OP.max,
                                            apply_absolute_value=True)

                def slab_round(r, xm):
                    lo = 4 * r
                    hi = min(NBIN, lo + 4)
                    psl = AXP.tile([128, 512], F32, tag="axp")
                    for b in range(lo, hi):
                        bs = slice(128 * b, 128 * b + 128)
                        ls = slice(128 * (b - lo), 128 * (b - lo) + 128)
                        nc.tensor.matmul(psl[:, ls], slabT[:, 0, bs],
                                         slabT[:, 0, bs], start=True,
                                         stop=False)
                        nc.tensor.matmul(psl[:, ls], slabT[:, 1, bs],
                                         slabT[:, 1, bs], start=False,
                                         stop=True)
                    w = 128 * (hi - lo)
                    nc.vector.tensor_mul(xm[:, 128 * lo:128 * lo + w],
                                         psl[:, 0:w],
                                         l2m[:, 128 * lo:128 * lo + w])

                def slab_poly(xm):
                    ut = SW.tile([128, SLABW], BF16, tag="ut")
                    nc.vector.tensor_mul(ut, xm, xm)
                    pA = SW.tile([128, SLABW], BF16, tag="pA")
                    nc.vector.tensor_scalar(
                        out=pA, in0=ut, scalar1=3.0 / 40.0, scalar2=1.0 / 6.0,
                        op0=OP.mult, op1=OP.add)
                    pD = SW.tile([128, SLABW], BF16, tag="pD")
                    nc.vector.tensor_mul(pD, pA, ut)
                    xD = SW.tile([128, SLABW], BF16, tag="xD")
                    nc.vector.tensor_mul(xD, xm, pD)
                    sS = SW.tile([128, SLABW], BF16, tag="sS")
                    nc.vector.tensor_add(sS, xm, xD)
                    a2 = SW.tile([128, SLABW], BF16, tag="a2")
                    nc.scalar.activation(a2, sS, AF.Square, bias=hp[:, 0:1],
                                         scale=-1.0)
                    zj = SW.tile([128, SLABW], BF16, tag="zj")
                    nc.vector.tensor_mul(zj, a2, l2m)
                    nc.vector.tensor_reduce(out1[:, 45:46], zj, axis=AX.X,
                                            op=OP.add)
                    nc.vector.tensor_reduce(out1[:, 46:47], xm, axis=AX.X,
                                            op=OP.max,
                                            apply_absolute_value=True)

                fillers = []
                xm = SW.tile([128, SLABW], BF16, tag="xm")
                for t in range(8):
                    if t == 7:
                        # t7: interleave pair-(6,7) colsums as X ranges land
                        ch = _chunks(7)
                        sl = _slots(7)
                        ex = _exports(6)
                        chunk(7, ch[0][0], ch[0][1], sl[0], fillers, 99)
                        chunk(7, ch[1][0], ch[1][1], sl[1],
                              [lambda: [cs_mm(6, m, off)
                                        for m, off in ex if off < 1536]], 99)
                        chunk(7, ch[2][0], ch[2][1], sl[2],
                              [lambda: [cs_mm(6, m, off)
                                        for m, off in ex
                                        if 1536 <= off < 3072]], 99)
                        rest = [(m, off) for m, off in ex if off >= 3072]
                        for i, (m, off) in enumerate(rest):
                            cs_mm(6, m, off, last=(i == len(rest) - 1))
                        continue
                    ch = _chunks(t)
                    sl = _slots(t)
                    for i, (c0, w) in enumerate(ch):
                        chunk(t, c0, w, sl[i], fillers)
                    if t == 0:
                        fillers.append(lambda: slab_round(0, xm))
                        fillers.append(lambda: slab_round(1, xm))
                    if t == 1:
                        fillers.append(lambda: slab_poly(xm))
                    if t in (1, 3, 5):
                        ex = _exports(t - 1)
                        for j in (0, 3, 5):
                            grp = ex[j:j + (3 if j == 0 else 2)]
                            fillers.append(
                                lambda tt=t - 1, g=list(grp): [
                                    cs_mm(tt, m, off) for m, off in g])
                    for g in GUARD_SCHED.get(t, []):
                        fillers.append(lambda gg=g: guard_piece(gg))
                nc.vector.tensor_copy(cso, csum[0:8, :])

            nc.sync.dma_start(out=out1d[:, :], in_=out1)
            nc.sync.dma_start(out=out2d[:, :], in_=cso)
    nc.compile()
    return nc


_CACHE = {}


def _host_amc(amn, labels):
    sim = (amn @ amn.T).astype(np.float64)
    ang = np.arccos(np.clip(sim, -1 + 1e-7, 1 - 1e-7))
    lm = labels[:, None] == labels[None, :]
    od = ~np.eye(len(labels), dtype=bool)
    l1 = np.where((~lm) & od, np.maximum(0.0, MARGIN - ang) ** 2, 0.0).sum()
    l2 = np.where(lm & od, ang ** 2, 0.0).sum()
    return float(l1 + l2)


def kernel(am_features, projection1, projection2, labels):
    if "nc" not in _CACHE:
        _CACHE["nc"] = build()
    nc = _CACHE["nc"]

    import ml_dtypes
    amf = np.asarray(am_features, dtype=np.float64)
    p1 = np.asarray(projection1, dtype=np.float64)
    p2 = np.asarray(projection2, dtype=np.float64)
    lab = np.asarray(labels).astype(np.int64)

    p1n = p1 / np.linalg.norm(p1, axis=1, keepdims=True)
    p2n = p2 / np.linalg.norm(p2, axis=1, keepdims=True)
    amnorm = np.linalg.norm(amf, axis=1, keepdims=True)
    amn = amf / amnorm
    rn_full = (1.0 / amnorm[:, 0]).astype(np.float32)
    p1n32 = p1n.astype(np.float32)
    p2n32 = p2n.astype(np.float32)
    amT32 = np.ascontiguousarray(amf.T.astype(np.float32))

    aux8 = np.zeros((128, 8, 2, 128), np.float32)
    for m in range(8):
        aux8[:, m, :, m] = 1.0
    aux8 = aux8.astype(ml_dtypes.float8_e5m2)

    counts = np.bincount(np.clip(lab, 0, None), minlength=NCLS)
    class_rows = [np.where(lab == c)[0] for c in range(NCLS)]
    host_fallback = (counts.max() > 128 or NCLS > NCORES * NBIN
                     or lab.min() < 0 or lab.max() >= NCLS)

    in_maps = []
    rn_k_all = []
    for k in range(NCORES):
        r = -k * SHARD
        c1 = np.roll(p1n32, r, axis=0)[0:2560]
        c2 = np.roll(p2n32, r, axis=0)[0:2560]
        cfT = np.ascontiguousarray(
            np.concatenate([c1.T, c2.T], axis=1)).astype(ml_dtypes.bfloat16)
        amT = np.ascontiguousarray(
            np.roll(amT32, r, axis=1)[:, 0:2560]).astype(
                ml_dtypes.float8_e4m3fn)
        rn_k_all.append(np.roll(rn_full, r)[0:512].reshape(4, 128))
        slab = np.zeros((SLABW, DA), dtype=np.float32)
        slab[:, 0] = 1.0
        mask = np.zeros((128, SLABW), dtype=np.float32)
        if not host_fallback:
            for lb in range(NBIN):
                c = k * NBIN + lb
                if c >= NCLS:
                    continue
                rows = class_rows[c]
                n = len(rows)
                slab[lb * 128:lb * 128 + n] = amn[rows]
                m = np.ones((n, n), np.float32) - np.eye(n, dtype=np.float32)
                mask[:n, lb * 128:lb * 128 + n] = m
        in_maps.append({
            "cfT": cfT,
            "amT": amT,
            "slabT": np.ascontiguousarray(slab.T).astype(ml_dtypes.bfloat16),
            "l2mask": mask.astype(ml_dtypes.bfloat16),
            "aux8": aux8,
        })

    res = run_bass_kernel_spmd(nc, in_maps, core_ids=list(range(NCORES)))
    o1 = [np.asarray(res.results[k]["out1"], dtype=np.float64)
          for k in range(NCORES)]
    o2 = [np.asarray(res.results[k]["out2"], dtype=np.float64)
          for k in range(NCORES)]

    d = np.zeros(2 * N)
    for k in range(NCORES):
        for t in range(8):
            dall_t = o1[k][:, _slots(t)].sum(axis=1)
            g0 = (512 * k + 128 * (t % 4)) % N
            h = 0 if t < 4 else N
            d[h + g0:h + g0 + 128] += dall_t
        for c1 in (1, 2, 3, 4):
            g0 = (512 * c1 + 512 * k) % N
            d[g0:g0 + 512] += o2[k][c1 - 1, :]
        for c2 in (0, 1, 2, 3):
            g0 = (512 * c2 + 512 * k) % N
            d[N + g0:N + g0 + 512] += o2[k][4 + c2, :]
    d = XSCALE * d - 1.0
    if not np.isfinite(d).all() or (d <= 0).any():
        # pathological inputs (e.g. fp8 overflow): exact host recompute
        cf = np.concatenate([p1n, p2n], 0)
        d = np.empty(2 * N)
        for i0 in range(0, 2 * N, 512):
            d[i0:i0 + 512] = np.exp(cf[i0:i0 + 512] @ cf.T / T).sum(axis=1)
        d -= np.exp((cf * cf).sum(axis=1) / T)
    sum_log_d = np.log(d).sum()

    s = p1n + p2n
    cnt = counts.astype(np.float64)
    wden = 2.0 * cnt - 1.0
    G2 = np.zeros(NCLS)
    for c in range(NCLS):
        rows = class_rows[c]
        if len(rows):
            G2[c] = (s[rows].sum(axis=0) ** 2).sum()
    sclpos = (np.where(wden != 0, G2 / wden, 0.0)
              - np.where(wden != 0, 2.0 * cnt / wden, 0.0)).sum()
    loss1 = sum_log_d / (2 * N) - sclpos / (T * 2 * N)

    amc_l2 = sum(o1[k][:, 45].sum() for k in range(NCORES))
    slabmax = max(o1[k][:, 46].max() for k in range(NCORES))
    gbound = 0.0
    for k in range(NCORES):
        for it in range(4):
            gm = o1[k][:, NSLOT + 5 * it:NSLOT + 5 * it + 5].max(axis=1)
            gbound = max(gbound, (gm * rn_k_all[k][it]).max())
    gbound *= rn_full.max()
    if host_fallback or gbound > GUARD_SAFE or slabmax > POLY_SAFE:
        amc_total = _host_amc(amn, lab)
    else:
        amc_total = amc_l2
    loss2 = amc_total / 50.0
    return np.array(0.5 * loss1 + 0.5 * loss2, dtype=np.float32)


# revision 6
# speedup vs baseline: 1.0036x; 1.0036x over previous
"""AngularContrastiveLoss v4: symmetric half-band kernel, 8 cores.

v3 + stall fixes: slab/guard on a dedicated 1-bank psum pool (the stream's
double-buffer is never blocked by DVE consumers), diagonal masked by a PE
zero-matmul instead of a DVE mul (no cross-engine round trip inside the
stream), 512-wide first chunk + split input DMAs for a fast start, colsum
pairs and guard pieces spread between stream chunks via a filler queue,
tail colsums emitted as soon as the covering t7 chunk is exp'd.
Math identical to kernel3 (see its docstring); host assembly identical
except guard maxes arrive raw (20 pieces) and are scaled by rn on host.
"""
import numpy as np

import concourse.bass as bass
import concourse.bacc as bacc
import concourse.mybir as mybir
from concourse.tile import TileContext
from concourse.bass_utils import run_bass_kernel_spmd

F32 = mybir.dt.float32
BF16 = mybir.dt.bfloat16
FP8 = mybir.dt.float8e4
FP8W = mybir.dt.float8e5
PM = mybir.MatmulPerfMode
AF = mybir.ActivationFunctionType
OP = mybir.AluOpType
AX = mybir.AxisListType

NCORES = 8
N = 4096
DA = 256
NCLS = 50
T = 0.06
MARGIN = 0.5
HALF_PI = float(np.float32(np.pi / 2))
INV_T = float(np.float32(1.0 / T))
NEG5LN2 = float(np.float32(-5.0 * np.log(2.0)))
XSCALE = 32.0
SHARD = N // NCORES
NBIN = 7
SLABW = NBIN * 128
CW = 1536
ROWC = 4608
POLY_SAFE = 0.45
GUARD_SAFE = 0.80

CHUNKS0 = [(0, 512), (512, 1536), (2048, 1536), (3584, 1024)]
CHUNKS = [(0, 1536), (1536, 1536), (3072, 1536)]
CHUNKS7 = [(0, 1536), (1536, 1536), (3072, 1024), (4096, 512)]


def _chunks(t):
    if t == 0:
        return CHUNKS0
    return CHUNKS7 if t == 7 else CHUNKS


_SLOTBASE = {}
_n = 0
for _t in range(8):
    _SLOTBASE[_t] = _n
    _n += len(_chunks(_t))
NSLOT = _n  # 26


def _slots(t):
    return [_SLOTBASE[t] + i for i in range(len(_chunks(t)))]


def _exports(t):
    if t < 4:
        e = [(c1 - 1, 512 * c1) for c1 in (1, 2, 3)]
        e += [(4 + c2, 2560 + 512 * c2) for c2 in (0, 1, 2, 3)]
    else:
        e = [(c1 - 1, 512 * c1 - 512) for c1 in (1, 2, 3, 4)]
        e += [(4 + c2, 2048 + 512 * c2) for c2 in (1, 2, 3)]
    return e


# guard piece schedule: piece g = 5*it + pc; cols [512pc, 512pc+512)
GUARD_SCHED = {1: [0, 5, 10], 2: [15, 1, 6], 3: [11, 16, 2],
               4: [7, 12, 17, 3], 5: [8, 13, 14, 18], 6: [4, 9, 19]}


def build():
    nc = bacc.Bacc("TRN2", target_bir_lowering=False, debug=False,
                   num_devices=NCORES)
    cftd = nc.declare_dram_parameter("cfT", [128, 5120], BF16, isOutput=False)
    amtd = nc.declare_dram_parameter("amT", [DA, 2560], FP8, isOutput=False)
    slbd = nc.declare_dram_parameter("slabT", [DA, SLABW], BF16, isOutput=False)
    mskd = nc.declare_dram_parameter("l2mask", [128, SLABW], BF16, isOutput=False)
    ax8d = nc.declare_dram_parameter("aux8", [128, 8, 2, 128], FP8W, isOutput=False)
    out1d = nc.declare_dram_parameter("out1", [128, 48], F32, isOutput=True)
    out2d = nc.declare_dram_parameter("out2", [8, 512], F32, isOutput=True)

    with TileContext(nc) as tc:
        with tc.sbuf_pool(name="persist", bufs=1) as PP:
            cfT = PP.tile([128, 5120], BF16, tag="cfT")
            amT = PP.tile([128, 2, 2560], FP8, tag="amT")
            slabT = PP.tile([128, 2, SLABW], BF16, tag="slabT")
            l2m = PP.tile([128, SLABW], BF16, tag="l2m")
            ohm = PP.tile([128, 8, 2, 128], FP8W, tag="ohm")
            zer = PP.tile([128, 128], BF16, tag="zer")
            X = PP.tile([128, 8, ROWC], FP8W, tag="X")
            out1 = PP.tile([128, 48], F32, tag="out1")
            dacc = out1[:, 0:NSLOT]
            gt = out1[:, NSLOT:NSLOT + 20]
            hp = PP.tile([128, 1], F32, tag="hp")
            b5 = PP.tile([128, 1], F32, tag="b5")
            warm = PP.tile([128, 1], F32, tag="warm")
            cso = PP.tile([8, 512], F32, tag="cso")

            nc.vector.memset(b5, NEG5LN2)
            nc.vector.memset(hp, HALF_PI)
            nc.vector.memset(zer, 0.0)
            nc.scalar.activation(warm, hp, AF.Exp)

            nc.sync.dma_start(out=cfT[:, 0:512], in_=cftd[:, 0:512])
            nc.sync.dma_start(out=cfT[:, 512:1536], in_=cftd[:, 512:1536])
            nc.sync.dma_start(out=cfT[:, 1536:2048], in_=cftd[:, 1536:2048])
            nc.sync.dma_start(out=cfT[:, 2048:3584], in_=cftd[:, 2048:3584])
            nc.sync.dma_start(out=ohm, in_=ax8d[:, :, :, :])
            nc.sync.dma_start(
                out=amT[:, :, 0:1536],
                in_=amtd[:, 0:1536].rearrange("(c p) w -> p c w", p=128))
            nc.sync.dma_start(out=cfT[:, 3584:5120], in_=cftd[:, 3584:5120])
            nc.sync.dma_start(
                out=slabT,
                in_=slbd[:, :].rearrange("(c p) w -> p c w", p=128))
            nc.sync.dma_start(
                out=amT[:, :, 1536:2560],
                in_=amtd[:, 1536:2560].rearrange("(c p) w -> p c w", p=128))
            nc.sync.dma_start(out=l2m, in_=mskd[:, :])

            with tc.psum_pool(name="stream", bufs=2) as SP, \
                 tc.psum_pool(name="cspool", bufs=1) as CS, \
                 tc.psum_pool(name="auxp", bufs=1) as AXP, \
                 tc.sbuf_pool(name="slw", bufs=2) as SW:

                csum = CS.tile([128, 512], F32, tag="csum")
                cs_state = {"first": True}

                def cs_mm(t, m, off, last=False):
                    nc.tensor.matmul(
                        csum, ohm[:, m, :, :], X[:, t:t + 2, off:off + 512],
                        start=cs_state["first"], stop=last,
                        perf_mode=PM.DoubleRow, skip_group_check=True)
                    cs_state["first"] = False

                def chunk(t, c0, w, slot, fillers, max_pop=2):
                    base = 0 if t < 4 else 512
                    lt = 128 * t if t < 4 else 2560 + 128 * (t - 4)
                    lhs = cfT[:, lt:lt + 128]
                    dcol = lt - base
                    ps = SP.tile([128, CW], F32, tag="ps")
                    for s in range(w // 512):
                        nc.tensor.matmul(
                            ps[:, 512 * s:512 * s + 512], lhs,
                            cfT[:, base + c0 + 512 * s:base + c0 + 512 * s + 512],
                            start=True, stop=True)
                    if c0 <= dcol < c0 + w:
                        off = dcol - c0
                        nc.tensor.matmul(ps[:, off:off + 128], lhs, zer,
                                         start=True, stop=True,
                                         skip_group_check=True)
                    n = 0
                    while fillers and n < max_pop:
                        fillers.pop(0)()
                        n += 1
                    nc.scalar.activation(
                        X[:, t, c0:c0 + w], ps[:, 0:w], AF.Exp,
                        scale=INV_T, bias=b5[:, 0:1],
                        accum_out=dacc[:, slot:slot + 1])

                def guard_piece(g):
                    it, pc = g // 5, g % 5
                    gp = AXP.tile([128, 512], F32, tag="axp")
                    nc.tensor.matmul(
                        gp, amT[:, :, 128 * it:128 * it + 128],
                        amT[:, :, 512 * pc:512 * pc + 512],
                        start=True, stop=True, perf_mode=PM.DoubleRow)
                    if pc == 0:
                        off = 128 * it
                        nc.tensor.matmul(gp[:, off:off + 128],
                                         cfT[:, 0:128], zer,
                                         start=True, stop=True,
                                         skip_group_check=True)
                    nc.vector.tensor_reduce(gt[:, g:g + 1], gp, axis=AX.X,
                                            op=OP.max,
                                            apply_absolute_value=True)

                def slab_round(r, xm):
                    lo = 4 * r
                    hi = min(NBIN, lo + 4)
                    psl = AXP.tile([128, 512], F32, tag="axp")
                    for b in range(lo, hi):
                        bs = slice(128 * b, 128 * b + 128)
                        ls = slice(128 * (b - lo), 128 * (b - lo) + 128)
                        nc.tensor.matmul(psl[:, ls], slabT[:, 0, bs],
                                         slabT[:, 0, bs], start=True,
                                         stop=False)
                        nc.tensor.matmul(psl[:, ls], slabT[:, 1, bs],
                                         slabT[:, 1, bs], start=False,
                                         stop=True)
                    w = 128 * (hi - lo)
                    nc.vector.tensor_mul(xm[:, 128 * lo:128 * lo + w],
                                         psl[:, 0:w],
                                         l2m[:, 128 * lo:128 * lo + w])

                def slab_poly(xm):
                    ut = SW.tile([128, SLABW], BF16, tag="ut")
                    nc.vector.tensor_mul(ut, xm, xm)
                    pA = SW.tile([128, SLABW], BF16, tag="pA")
                    nc.vector.tensor_scalar(
                        out=pA, in0=ut, scalar1=3.0 / 40.0, scalar2=1.0 / 6.0,
                        op0=OP.mult, op1=OP.add)
                    pD = SW.tile([128, SLABW], BF16, tag="pD")
                    nc.vector.tensor_mul(pD, pA, ut)
                    xD = SW.tile([128, SLABW], BF16, tag="xD")
                    nc.vector.tensor_mul(xD, xm, pD)
                    sS = SW.tile([128, SLABW], BF16, tag="sS")
                    nc.vector.tensor_add(sS, xm, xD)
                    a2 = SW.tile([128, SLABW], BF16, tag="a2")
                    nc.scalar.activation(a2, sS, AF.Square, bias=hp[:, 0:1],
                                         scale=-1.0)
                    zj = SW.tile([128, SLABW], BF16, tag="zj")
                    nc.vector.tensor_mul(zj, a2, l2m)
                    nc.vector.tensor_reduce(out1[:, 46:47], zj, axis=AX.X,
                                            op=OP.add)
                    nc.vector.tensor_reduce(out1[:, 47:48], xm, axis=AX.X,
                                            op=OP.max,
                                            apply_absolute_value=True)

                fillers = []
                xm = SW.tile([128, SLABW], BF16, tag="xm")
                for t in range(8):
                    if t == 7:
                        # t7: interleave pair-(6,7) colsums as X ranges land
                        ch = _chunks(7)
                        sl = _slots(7)
                        ex = _exports(6)
                        chunk(7, ch[0][0], ch[0][1], sl[0], fillers, 99)
                        chunk(7, ch[1][0], ch[1][1], sl[1],
                              [lambda: [cs_mm(6, m, off)
                                        for m, off in ex if off < 1536]], 99)
                        chunk(7, ch[2][0], ch[2][1], sl[2],
                              [lambda: [cs_mm(6, m, off)
                                        for m, off in ex
                                        if 1536 <= off < 3072]], 99)
                        rest = [(m, off) for m, off in ex if off >= 3072]
                        for i, (m, off) in enumerate(rest):
                            cs_mm(6, m, off, last=(i == len(rest) - 1))
                        continue
                    ch = _chunks(t)
                    sl = _slots(t)
                    for i, (c0, w) in enumerate(ch):
                        chunk(t, c0, w, sl[i], fillers)
                    if t == 0:
                        fillers.append(lambda: slab_round(0, xm))
                        fillers.append(lambda: slab_round(1, xm))
                    if t == 1:
                        fillers.append(lambda: slab_poly(xm))
                    if t in (1, 3, 5):
                        ex = _exports(t - 1)
                        for j in (0, 3, 5):
                            grp = ex[j:j + (3 if j == 0 else 2)]
                            fillers.append(
                                lambda tt=t - 1, g=list(grp): [
                                    cs_mm(tt, m, off) for m, off in g])
                    for g in GUARD_SCHED.get(t, []):
                        fillers.append(lambda gg=g: guard_piece(gg))
                nc.vector.tensor_copy(cso, csum[0:8, :])

            nc.sync.dma_start(out=out1d[:, :], in_=out1)
            nc.sync.dma_start(out=out2d[:, :], in_=cso)
    nc.compile()
    return nc


_CACHE = {}


def _host_amc(amn, labels):
    sim = (amn @ amn.T).astype(np.float64)
    ang = np.arccos(np.clip(sim, -1 + 1e-7, 1 - 1e-7))
    lm = labels[:, None] == labels[None, :]
    od = ~np.eye(len(labels), dtype=bool)
    l1 = np.where((~lm) & od, np.maximum(0.0, MARGIN - ang) ** 2, 0.0).sum()
    l2 = np.where(lm & od, ang ** 2, 0.0).sum()
    return float(l1 + l2)


def kernel(am_features, projection1, projection2, labels):
    if "nc" not in _CACHE:
        _CACHE["nc"] = build()
    nc = _CACHE["nc"]

    import ml_dtypes
    amf = np.asarray(am_features, dtype=np.float64)
    p1 = np.asarray(projection1, dtype=np.float64)
    p2 = np.asarray(projection2, dtype=np.float64)
    lab = np.asarray(labels).astype(np.int64)

    p1n = p1 / np.linalg.norm(p1, axis=1, keepdims=True)
    p2n = p2 / np.linalg.norm(p2, axis=1, keepdims=True)
    amnorm = np.linalg.norm(amf, axis=1, keepdims=True)
    amn = amf / amnorm
    rn_full = (1.0 / amnorm[:, 0]).astype(np.float32)
    p1n32 = p1n.astype(np.float32)
    p2n32 = p2n.astype(np.float32)
    amT32 = np.ascontiguousarray(amf.T.astype(np.float32))

    aux8 = np.zeros((128, 8, 2, 128), np.float32)
    for m in range(8):
        aux8[:, m, :, m] = 1.0
    aux8 = aux8.astype(ml_dtypes.float8_e5m2)

    counts = np.bincount(np.clip(lab, 0, None), minlength=NCLS)
    class_rows = [np.where(lab == c)[0] for c in range(NCLS)]
    host_fallback = (counts.max() > 128 or NCLS > NCORES * NBIN
                     or lab.min() < 0 or lab.max() >= NCLS)

    in_maps = []
    rn_k_all = []
    for k in range(NCORES):
        r = -k * SHARD
        c1 = np.roll(p1n32, r, axis=0)[0:2560]
        c2 = np.roll(p2n32, r, axis=0)[0:2560]
        cfT = np.ascontiguousarray(
            np.concatenate([c1.T, c2.T], axis=1)).astype(ml_dtypes.bfloat16)
        amT = np.ascontiguousarray(
            np.roll(amT32, r, axis=1)[:, 0:2560]).astype(
                ml_dtypes.float8_e4m3fn)
        rn_k_all.append(np.roll(rn_full, r)[0:512].reshape(4, 128))
        slab = np.zeros((SLABW, DA), dtype=np.float32)
        slab[:, 0] = 1.0
        mask = np.zeros((128, SLABW), dtype=np.float32)
        if not host_fallback:
            for lb in range(NBIN):
                c = k * NBIN + lb
                if c >= NCLS:
                    continue
                rows = class_rows[c]
                n = len(rows)
                slab[lb * 128:lb * 128 + n] = amn[rows]
                m = np.ones((n, n), np.float32) - np.eye(n, dtype=np.float32)
                mask[:n, lb * 128:lb * 128 + n] = m
        in_maps.append({
            "cfT": cfT,
            "amT": amT,
            "slabT": np.ascontiguousarray(slab.T).astype(ml_dtypes.bfloat16),
            "l2mask": mask.astype(ml_dtypes.bfloat16),
            "aux8": aux8,
        })

    res = run_bass_kernel_spmd(nc, in_maps, core_ids=list(range(NCORES)))
    o1 = [np.asarray(res.results[k]["out1"], dtype=np.float64)
          for k in range(NCORES)]
    o2 = [np.asarray(res.results[k]["out2"], dtype=np.float64)
          for k in range(NCORES)]

    d = np.zeros(2 * N)
    for k in range(NCORES):
        for t in range(8):
            dall_t = o1[k][:, _slots(t)].sum(axis=1)
            g0 = (512 * k + 128 * (t % 4)) % N
            h = 0 if t < 4 else N
            d[h + g0:h + g0 + 128] += dall_t
        for c1 in (1, 2, 3, 4):
            g0 = (512 * c1 + 512 * k) % N
            d[g0:g0 + 512] += o2[k][c1 - 1, :]
        for c2 in (0, 1, 2, 3):
            g0 = (512 * c2 + 512 * k) % N
            d[N + g0:N + g0 + 512] += o2[k][4 + c2, :]
    d = XSCALE * d - 1.0
    if not np.isfinite(d).all() or (d <= 0).any():
        # pathological inputs (e.g. fp8 overflow): exact host recompute
        cf = np.concatenate([p1n, p2n], 0)
        d = np.empty(2 * N)
        for i0 in range(0, 2 * N, 512):
            d[i0:i0 + 512] = np.exp(cf[i0:i0 + 512] @ cf.T / T).sum(axis=1)
        d -= np.exp((cf * cf).sum(axis=1) / T)
    sum_log_d = np.log(d).sum()

    s = p1n + p2n
    cnt = counts.astype(np.float64)
    wden = 2.0 * cnt - 1.0
    G2 = np.zeros(NCLS)
    for c in range(NCLS):
        rows = class_rows[c]
        if len(rows):
            G2[c] = (s[rows].sum(axis=0) ** 2).sum()
    sclpos = (np.where(wden != 0, G2 / wden, 0.0)
              - np.where(wden != 0, 2.0 * cnt / wden, 0.0)).sum()
    loss1 = sum_log_d / (2 * N) - sclpos / (T * 2 * N)

    amc_l2 = sum(o1[k][:, 46].sum() for k in range(NCORES))
    slabmax = max(o1[k][:, 47].max() for k in range(NCORES))
    gbound = 0.0
    for k in range(NCORES):
        for it in range(4):
            gm = o1[k][:, NSLOT + 5 * it:NSLOT + 5 * it + 5].max(axis=1)
            gbound = max(gbound, (gm * rn_k_all[k][it]).max())
    gbound *= rn_full.max()
    if host_fallback or gbound > GUARD_SAFE or slabmax > POLY_SAFE:
        amc_total = _host_amc(amn, lab)
    else:
        amc_total = amc_l2
    loss2 = amc_total / 50.0
    return np.array(0.5 * loss1 + 0.5 * loss2, dtype=np.float32)
